# revision 6
# baseline (speedup 1.0000x reference)
"""Trainium2 Bass kernel for CrossLayerSharedZOlmoeSparseMoeBlock.

Strategy (expert-parallel, 8 cores):
  host: full routing math in fp32 numpy (predictor MLP + gumbel argmax +
        router softmax + top-8-of-16) -> comb [T, E]; per-expert token
        index lists; experts paired best-with-worst by load and assigned
        2 slots/core (slot sizes are compile-time constants = max over
        cores); token buffers gathered/compacted per slot in bf16.
  device (one kernel launch): per core, per slot: gate/up/down matmuls
        in bf16 (fp32 PSUM accumulate), silu*up fused at PSUM eviction,
        routing weight applied on-chip at down-proj eviction. Exact token
        counts (no 128-padding of the moving dim).
  host: scatter-add compact fp32 outputs into y.

bf16 matmuls run at 1 cycle/row on the PE (vs ~1.8 for f32r's
fp32_mode=HIGH lowering) and halve weight/activation DMA traffic.
Aggregate rel err ~4e-3 (tolerance 2e-2).
"""
import contextlib
import ctypes
import math
import os
import sys
import types

import ml_dtypes
import numpy as np

sys.path.insert(0, "/opt/trn_rl_repo")

# ---------------------------------------------------------------------------
# NTFF profile hook shim (antenv.axon_hooks is absent in this image; bass's
# trace=True path imports it). Lets us read HW exec time via neuron profile.
# ---------------------------------------------------------------------------
_SO_PATH = "/opt/axon/libaxon_pjrt.so"


def _ntff_profile_via_ctypes(so_path):
    try:
        lib = ctypes.CDLL(so_path)
    except OSError:
        return None
    if not hasattr(lib, "axon_start_nrt_profile"):
        return None
    lib.axon_start_nrt_profile.argtypes = [ctypes.POINTER(ctypes.c_int64), ctypes.c_size_t]
    lib.axon_start_nrt_profile.restype = ctypes.c_int64
    lib.axon_stop_nrt_profile.argtypes = [ctypes.c_char_p]
    lib.axon_stop_nrt_profile.restype = ctypes.c_int64

    @contextlib.contextmanager
    def _hook(output_dir, device_ids):
        import jax

        jax.devices()
        if device_ids:
            ids = (ctypes.c_int64 * len(device_ids))(*device_ids)
            rc = lib.axon_start_nrt_profile(ids, len(device_ids))
        else:
            rc = lib.axon_start_nrt_profile(None, 0)
        if rc != 0:
            raise RuntimeError(f"axon_start_nrt_profile rc={rc}")
        try:
            yield
        finally:
            n = lib.axon_stop_nrt_profile(str(output_dir).encode())
            print(f"ntff profile: {n} file(s) -> {output_dir}", file=sys.stderr)

    return _hook


def _install_hook():
    if "antenv.axon_hooks" in sys.modules:
        return
    mod = types.ModuleType("antenv.axon_hooks")
    _h = [_ntff_profile_via_ctypes(_SO_PATH)]
    mod.get_axon_ntff_profile_hook = lambda: _h[0]
    mod.set_axon_ntff_profile_hook = lambda h: _h.__setitem__(0, h)
    sys.modules["antenv.axon_hooks"] = mod
    try:
        import antenv

        antenv.axon_hooks = mod
    except ImportError:
        pass


_install_hook()

import concourse.mybir as mybir  # noqa: E402
import concourse.tile as tile  # noqa: E402
from concourse import bacc  # noqa: E402
from concourse.bass_utils import run_bass_kernel_spmd  # noqa: E402

F32 = mybir.dt.float32
BF16 = mybir.dt.bfloat16
ALU = mybir.AluOpType
ACTF = mybir.ActivationFunctionType

# problem shapes (hardcoded per contest rules)
B, S, H = 1, 2048, 2048
T = B * S
E, F = 16, 1024
Z, M = 8, 512
TOP_K = 8
EPS = 1e-10
TAU = 1.0
N_CORES = 8
P = 128
KH = H // P          # 16 contraction chunks over H
MF = F // P          # 8 F tiles for gate/up
KF = F // P          # 8 contraction chunks over F
HS = H // 512        # 4 moving slices of 512 for down-proj
CAP = 1536           # max tokens per slot (SBUF budget guard)

TRACE = bool(int(os.environ.get("BASSMOE_TRACE", "0")))
BF = ml_dtypes.bfloat16

_timings = {}
_build_cache = {}


def _slices(C):
    # progressive first slices so compute can start before the full token
    # buffer lands
    out, off = [], 0
    for w in (128, 256):
        if off >= C:
            return out
        cw = min(w, C - off)
        out.append((off, cw))
        off += cw
    while off < C:
        cw = min(512, C - off)
        out.append((off, cw))
        off += cw
    return out


# ---------------------------------------------------------------------------
# K2: expert kernel. sizes = per-slot token counts (compile-time).
# ---------------------------------------------------------------------------
def build_k2(sizes):
    nc = bacc.Bacc(None, target_bir_lowering=False)
    ins, outs = [], []
    for s, C in enumerate(sizes):
        CC = (C + P - 1) // P
        ins.append((
            nc.dram_tensor(f"xg{s}", [P, KH, C], BF16, kind="ExternalInput"),
            nc.dram_tensor(f"wg{s}", [MF, P, KH, P], BF16, kind="ExternalInput"),
            nc.dram_tensor(f"wu{s}", [MF, P, KH, P], BF16, kind="ExternalInput"),
            nc.dram_tensor(f"wd{s}", [P, KF, HS, 512], BF16, kind="ExternalInput"),
            nc.dram_tensor(f"wv{s}", [P, CC], F32, kind="ExternalInput"),
        ))
        outs.append(nc.dram_tensor(f"out{s}", [CC, P, HS * 512], F32,
                                   kind="ExternalOutput"))

    with tile.TileContext(nc) as tc:
        with tc.tile_pool(name="xg", bufs=1) as xg_pool, \
             tc.tile_pool(name="act", bufs=2) as act_pool, \
             tc.tile_pool(name="wgu", bufs=2) as wgu_pool, \
             tc.tile_pool(name="wd", bufs=1) as wd_pool, \
             tc.tile_pool(name="wvp", bufs=2) as wv_pool, \
             tc.tile_pool(name="tmp", bufs=3) as tmp_pool, \
             tc.tile_pool(name="ev", bufs=3) as ev_pool, \
             tc.tile_pool(name="psg", bufs=2, space="PSUM") as psg, \
             tc.tile_pool(name="psu", bufs=2, space="PSUM") as psu, \
             tc.tile_pool(name="psd", bufs=2, space="PSUM") as psd:
            # PE warmup (HAM unthrottle) while the first DMAs land. The warm
            # tile is the wd-pool buffer: wd prefetches then wait for warmup
            # reads instead of racing the critical xg/wg DMAs at t=0.
            warm = wd_pool.tile([P, KF, HS, 512], BF16, name="wd")
            nc.vector.memset(warm[:, 0, 0, :], 0.0)
            for i in range(14):
                wps = (psg if i % 2 == 0 else psu).tile(
                    [P, 512], F32, name=("pg" if i % 2 == 0 else "pu"))
                nc.tensor.matmul(out=wps[:], lhsT=warm[:, 0, 0, :P],
                                 rhs=warm[:, 0, 0, :], start=True, stop=True)

            for s, C in enumerate(sizes):
                xgD, wgD, wuD, wdD, wvD = ins[s]
                outD = outs[s]
                CC = (C + P - 1) // P
                CS = _slices(C)

                # token buffer: k-halves split across two queues, col-sliced
                # so the first matmuls start as soon as their region lands
                xg = xg_pool.tile([P, KH, C], BF16, name="xg")
                for (c0, cw) in CS:
                    nc.sync.dma_start(out=xg[:, :KH // 2, c0:c0 + cw],
                                      in_=xgD[:, :KH // 2, c0:c0 + cw])
                    nc.gpsimd.dma_start(out=xg[:, KH // 2:, c0:c0 + cw],
                                        in_=xgD[:, KH // 2:, c0:c0 + cw])
                wv = wv_pool.tile([P, CC], F32, name="wv")
                nc.scalar.dma_start(out=wv[:], in_=wvD[:])

                actT = act_pool.tile([P, KF, C], BF16, name="actT")
                wd = wd_pool.tile([P, KF, HS, 512], BF16, name="wd")
                nc.scalar.dma_start(out=wd[:], in_=wdD[:])
                for m in range(MF):
                    wg = wgu_pool.tile([P, KH, P], BF16, name="wg")
                    nc.scalar.dma_start(out=wg[:], in_=wgD[m])
                    wu = wgu_pool.tile([P, KH, P], BF16, name="wu")
                    nc.scalar.dma_start(out=wu[:], in_=wuD[m])
                    for (c0, cw) in CS:
                        pg = psg.tile([P, 512], F32, name="pg")[:, :cw]
                        pu = psu.tile([P, 512], F32, name="pu")[:, :cw]
                        for k in range(KH):
                            nc.tensor.matmul(
                                out=pg[:], lhsT=wg[:, k, :],
                                rhs=xg[:, k, c0:c0 + cw],
                                start=(k == 0), stop=(k == KH - 1))
                        for k in range(KH):
                            nc.tensor.matmul(
                                out=pu[:], lhsT=wu[:, k, :],
                                rhs=xg[:, k, c0:c0 + cw],
                                start=(k == 0), stop=(k == KH - 1))
                        sg = tmp_pool.tile([P, 512], F32, name="sg")[:, :cw]
                        nc.scalar.activation(out=sg[:], in_=pg[:], func=ACTF.Silu,
                                             bias=0.0, scale=1.0)
                        nc.vector.tensor_tensor(
                            out=actT[:, m, c0:c0 + cw], in0=sg[:], in1=pu[:],
                            op=ALU.mult)

                # down projection; routing weight applied at eviction; one
                # batched output DMA per 128-token chunk
                for cc in range(CC):
                    rows = min(P, C - cc * P)
                    ev = ev_pool.tile([P, HS * 512], F32, name="ev")
                    for hs in range(HS):
                        pd = psd.tile([P, 512], F32, name="pd")
                        for k in range(KF):
                            nc.tensor.matmul(
                                out=pd[:rows, :],
                                lhsT=actT[:, k, cc * P:cc * P + rows],
                                rhs=wd[:, k, hs, :],
                                start=(k == 0), stop=(k == KF - 1))
                        nc.vector.tensor_scalar(
                            out=ev[:rows, hs * 512:(hs + 1) * 512],
                            in0=pd[:rows, :],
                            scalar1=wv[:rows, cc:cc + 1], scalar2=None,
                            op0=ALU.mult)
                    evq = nc.sync if cc % 2 == 0 else nc.scalar
                    evq.dma_start(out=outD[cc, :rows, :],
                                  in_=ev[:rows, :])
    nc.compile()
    return nc


# ---------------------------------------------------------------------------
# host routing (exact fp32 replication of the reference)
# ---------------------------------------------------------------------------
def _host_routing(x, gumbel_u, W1, b1, W2, b2, gate_w, U, alpha):
    h1 = x @ W1.T + b1
    h1 *= 1.0 / (1.0 + np.exp(-h1))                       # silu
    zl = h1 @ W2.T + b2
    g = -np.log(-np.log(gumbel_u + EPS) + EPS)
    s = (zl + g) / TAU
    s -= s.max(-1, keepdims=True)
    es = np.exp(s)
    soft = es / es.sum(-1, keepdims=True)
    hard = np.zeros_like(soft)
    hard[np.arange(T), soft.argmax(-1)] = 1.0
    z = (hard + soft) - soft                              # straight-through
    rl = x @ gate_w.T + np.float32(alpha) * (z @ U)
    rl -= rl.max(-1, keepdims=True)
    er = np.exp(rl)
    rw = er / er.sum(-1, keepdims=True)
    order = np.argsort(-rw, axis=1, kind="stable")[:, :TOP_K]
    topw = np.take_along_axis(rw, order, axis=1)
    return order, topw


def kernel(hidden_states, gumbel_u, W1, b1, W2, b2, gate_w, U, alpha, Wg, Wu, Wd):
    import time as _time

    t_start = _time.time()
    x = np.ascontiguousarray(np.asarray(hidden_states, np.float32).reshape(T, H))

    # ---- routing on host ----
    t0 = _time.time()
    order, topw = _host_routing(
        x, np.asarray(gumbel_u, np.float32),
        np.asarray(W1, np.float32), np.asarray(b1, np.float32),
        np.asarray(W2, np.float32), np.asarray(b2, np.float32),
        np.asarray(gate_w, np.float32), np.asarray(U, np.float32), alpha)
    idxs = [None] * E
    wvals = [None] * E
    tok = np.arange(T)
    for e in range(E):
        rows, cols = np.nonzero(order == e)
        idxs[e] = rows
        wvals[e] = topw[rows, cols].astype(np.float32)
    _timings["routing"] = _time.time() - t0

    # ---- pack pieces into 8 cores x nslots ----
    t0 = _time.time()
    pieces = []
    for e in range(E):
        c = len(idxs[e])
        nparts = max(1, math.ceil(c / CAP))
        base, rem = divmod(c, nparts)
        off = 0
        for i in range(nparts):
            ln = base + (1 if i < rem else 0)
            pieces.append((e, off, ln))
            off += ln

    def cost(ln):
        return 256 * ln + 16384 * math.ceil(ln / P)

    pieces.sort(key=lambda p: -p[2])
    loads = [0] * N_CORES
    assign = [[] for _ in range(N_CORES)]
    for pc in pieces:
        c = min(range(N_CORES), key=lambda i: loads[i])
        assign[c].append(pc)
        loads[c] += cost(pc[2])
    nslots = max(len(a) for a in assign)
    for a in assign:
        a.sort(key=lambda p: -p[2])
        while len(a) < nslots:
            a.append((0, 0, 0))                            # dummy slot
    sizes = [max(P, max(assign[c][i][2] for c in range(N_CORES)))
             for i in range(nslots)]

    # ---- weight/activation prep (bf16, transposed+interleaved) ----
    xT = np.ascontiguousarray(
        x.reshape(T, KH, P).transpose(2, 1, 0).astype(BF))   # [128, 16, T]
    WgB = np.asarray(Wg, np.float32).astype(BF)
    WuB = np.asarray(Wu, np.float32).astype(BF)
    WdB = np.asarray(Wd, np.float32).astype(BF)
    # wgt[e,m,p,k,j] = Wg[e, m*128+j, k*128+p]
    WgT = np.ascontiguousarray(
        WgB.reshape(E, MF, P, KH, P).transpose(0, 1, 4, 3, 2))
    WuT = np.ascontiguousarray(
        WuB.reshape(E, MF, P, KH, P).transpose(0, 1, 4, 3, 2))
    # wdt[e,p,k,hs,j] = Wd[e, hs*512+j, k*128+p]
    WdT = np.ascontiguousarray(
        WdB.reshape(E, HS, 512, KF, P).transpose(0, 4, 3, 1, 2))

    in_maps = []
    for c in range(N_CORES):
        m = {}
        for si in range(nslots):
            e, off, ln = assign[c][si]
            Csz = sizes[si]
            CC = (Csz + P - 1) // P
            xg = np.zeros((P, KH, Csz), BF)
            wvp = np.zeros((CC * P,), np.float32)
            if ln > 0:
                sel = idxs[e][off:off + ln]
                xg[:, :, :ln] = xT[:, :, sel]
                wvp[:ln] = wvals[e][off:off + ln]
            m[f"xg{si}"] = xg
            m[f"wg{si}"] = WgT[e]
            m[f"wu{si}"] = WuT[e]
            m[f"wd{si}"] = WdT[e]
            m[f"wv{si}"] = np.ascontiguousarray(wvp.reshape(CC, P).T)
        in_maps.append(m)
    _timings["dispatch"] = _time.time() - t0

    t0 = _time.time()
    key = tuple(sizes)
    nc2 = _build_cache.get(key)
    if nc2 is None:
        nc2 = build_k2(sizes)
        _build_cache[key] = nc2
    _timings["k2_build"] = _time.time() - t0

    t0 = _time.time()
    res2 = run_bass_kernel_spmd(nc2, in_maps, list(range(N_CORES)), trace=TRACE)
    _timings["k2_run"] = _time.time() - t0
    if TRACE:
        _timings["k2_hw_ns"] = res2.exec_time_ns

    # ---- host combine (unshard) ----
    t0 = _time.time()
    y = np.zeros((T, H), np.float32)
    for c in range(N_CORES):
        for si in range(nslots):
            e, off, ln = assign[c][si]
            if ln == 0:
                continue
            oc = res2.results[c][f"out{si}"]             # [CC, 128, 2048]
            y[idxs[e][off:off + ln]] += oc.reshape(-1, H)[:ln]
    _timings["combine"] = _time.time() - t0
    _timings["total"] = _time.time() - t_start
    return y.reshape(B, S, H)


# revision 7
# speedup vs baseline: 1.0430x; 1.0430x over previous
"""Trainium2 Bass kernel for CrossLayerSharedZOlmoeSparseMoeBlock.

Strategy (expert-parallel, 8 cores):
  host: full routing math in fp32 numpy (predictor MLP + gumbel argmax +
        router softmax + top-8-of-16) -> comb [T, E]; per-expert token
        index lists; experts paired best-with-worst by load and assigned
        2 slots/core (slot sizes are compile-time constants = max over
        cores); token buffers gathered/compacted per slot in bf16.
  device (one kernel launch): per core, per slot: gate/up/down matmuls
        in bf16 (fp32 PSUM accumulate), silu*up fused at PSUM eviction,
        routing weight applied on-chip at down-proj eviction. Exact token
        counts (no 128-padding of the moving dim).
  host: scatter-add compact fp32 outputs into y.

bf16 matmuls run at 1 cycle/row on the PE (vs ~1.8 for f32r's
fp32_mode=HIGH lowering) and halve weight/activation DMA traffic.
Aggregate rel err ~4e-3 (tolerance 2e-2).
"""
import contextlib
import ctypes
import math
import os
import sys
import types

import ml_dtypes
import numpy as np

sys.path.insert(0, "/opt/trn_rl_repo")

# ---------------------------------------------------------------------------
# NTFF profile hook shim (antenv.axon_hooks is absent in this image; bass's
# trace=True path imports it). Lets us read HW exec time via neuron profile.
# ---------------------------------------------------------------------------
_SO_PATH = "/opt/axon/libaxon_pjrt.so"


def _ntff_profile_via_ctypes(so_path):
    try:
        lib = ctypes.CDLL(so_path)
    except OSError:
        return None
    if not hasattr(lib, "axon_start_nrt_profile"):
        return None
    lib.axon_start_nrt_profile.argtypes = [ctypes.POINTER(ctypes.c_int64), ctypes.c_size_t]
    lib.axon_start_nrt_profile.restype = ctypes.c_int64
    lib.axon_stop_nrt_profile.argtypes = [ctypes.c_char_p]
    lib.axon_stop_nrt_profile.restype = ctypes.c_int64

    @contextlib.contextmanager
    def _hook(output_dir, device_ids):
        import jax

        jax.devices()
        if device_ids:
            ids = (ctypes.c_int64 * len(device_ids))(*device_ids)
            rc = lib.axon_start_nrt_profile(ids, len(device_ids))
        else:
            rc = lib.axon_start_nrt_profile(None, 0)
        if rc != 0:
            raise RuntimeError(f"axon_start_nrt_profile rc={rc}")
        try:
            yield
        finally:
            n = lib.axon_stop_nrt_profile(str(output_dir).encode())
            print(f"ntff profile: {n} file(s) -> {output_dir}", file=sys.stderr)

    return _hook


def _install_hook():
    if "antenv.axon_hooks" in sys.modules:
        return
    mod = types.ModuleType("antenv.axon_hooks")
    _h = [_ntff_profile_via_ctypes(_SO_PATH)]
    mod.get_axon_ntff_profile_hook = lambda: _h[0]
    mod.set_axon_ntff_profile_hook = lambda h: _h.__setitem__(0, h)
    sys.modules["antenv.axon_hooks"] = mod
    try:
        import antenv

        antenv.axon_hooks = mod
    except ImportError:
        pass


_install_hook()

import concourse.mybir as mybir  # noqa: E402
import concourse.tile as tile  # noqa: E402
from concourse import bacc  # noqa: E402
from concourse.bass_utils import run_bass_kernel_spmd  # noqa: E402

F32 = mybir.dt.float32
BF16 = mybir.dt.bfloat16
ALU = mybir.AluOpType
ACTF = mybir.ActivationFunctionType

# problem shapes (hardcoded per contest rules)
B, S, H = 1, 2048, 2048
T = B * S
E, F = 16, 1024
Z, M = 8, 512
TOP_K = 8
EPS = 1e-10
TAU = 1.0
N_CORES = 8
P = 128
KH = H // P          # 16 contraction chunks over H
MF = F // P          # 8 F tiles for gate/up
KF = F // P          # 8 contraction chunks over F
HS = H // 512        # 4 moving slices of 512 for down-proj
CAP = 1536           # max tokens per slot (SBUF budget guard)

TRACE = bool(int(os.environ.get("BASSMOE_TRACE", "0")))
BF = ml_dtypes.bfloat16

_timings = {}
_build_cache = {}


def _slices(C):
    # progressive first slices so compute can start before the full token
    # buffer lands
    out, off = [], 0
    for w in (128, 256):
        if off >= C:
            return out
        cw = min(w, C - off)
        out.append((off, cw))
        off += cw
    while off < C:
        cw = min(512, C - off)
        out.append((off, cw))
        off += cw
    return out


# ---------------------------------------------------------------------------
# K2: expert kernel. sizes = per-slot token counts (compile-time).
# ---------------------------------------------------------------------------
def build_k2(sizes):
    nc = bacc.Bacc(None, target_bir_lowering=False)
    ins, outs = [], []
    for s, C in enumerate(sizes):
        CC = (C + P - 1) // P
        ins.append((
            nc.dram_tensor(f"xg{s}", [P, KH * C], BF16, kind="ExternalInput"),
            nc.dram_tensor(f"wg{s}", [MF, P, KH * P], BF16, kind="ExternalInput"),
            nc.dram_tensor(f"wu{s}", [MF, P, KH * P], BF16, kind="ExternalInput"),
            nc.dram_tensor(f"wd{s}", [P, KF * HS * 512], BF16,
                           kind="ExternalInput"),
            nc.dram_tensor(f"wv{s}", [P, CC], F32, kind="ExternalInput"),
        ))
        outs.append(nc.dram_tensor(f"out{s}", [CC, P, HS * 512], F32,
                                   kind="ExternalOutput"))

    with tile.TileContext(nc) as tc:
        with tc.tile_pool(name="xg", bufs=1) as xg_pool, \
             tc.tile_pool(name="act", bufs=2) as act_pool, \
             tc.tile_pool(name="wgu", bufs=2) as wgu_pool, \
             tc.tile_pool(name="wd", bufs=1) as wd_pool, \
             tc.tile_pool(name="wvp", bufs=2) as wv_pool, \
             tc.tile_pool(name="tmp", bufs=3) as tmp_pool, \
             tc.tile_pool(name="ev", bufs=3) as ev_pool, \
             tc.tile_pool(name="psg", bufs=2, space="PSUM") as psg, \
             tc.tile_pool(name="psu", bufs=2, space="PSUM") as psu, \
             tc.tile_pool(name="psd", bufs=2, space="PSUM") as psd:
            # PE warmup (HAM unthrottle) while the first DMAs land. The warm
            # tile is the wd-pool buffer: wd prefetches then wait for warmup
            # reads instead of racing the critical xg/wg DMAs at t=0.
            warm = wd_pool.tile([P, KF * HS * 512], BF16, name="wd")
            nc.vector.memset(warm[:, :512], 0.0)
            for i in range(8):
                wps = (psg if i % 2 == 0 else psu).tile(
                    [P, 512], F32, name=("pg" if i % 2 == 0 else "pu"))
                nc.tensor.matmul(out=wps[:], lhsT=warm[:, :P],
                                 rhs=warm[:, :512], start=True, stop=True)

            for s, C in enumerate(sizes):
                xgD, wgD, wuD, wdD, wvD = ins[s]
                outD = outs[s]
                CC = (C + P - 1) // P
                CS = _slices(C)

                # token buffer, slice-major: each slice is one contiguous
                # per-partition run (full DMA line rate), alternating queues;
                # the first (small) slices land fast so compute starts early
                xg = xg_pool.tile([P, KH * C], BF16, name="xg")
                for si, (c0, cw) in enumerate(CS):
                    o = KH * c0
                    q = nc.sync if si % 2 == 0 else nc.gpsimd
                    q.dma_start(out=xg[:, o:o + KH * cw],
                                in_=xgD[:, o:o + KH * cw])
                wv = wv_pool.tile([P, CC], F32, name="wv")
                nc.scalar.dma_start(out=wv[:], in_=wvD[:])

                actT = act_pool.tile([P, KF, C], BF16, name="actT")
                wd = wd_pool.tile([P, KF * HS * 512], BF16, name="wd")
                nc.scalar.dma_start(out=wd[:], in_=wdD[:])
                for m in range(MF):
                    wg = wgu_pool.tile([P, KH * P], BF16, name="wg")
                    nc.scalar.dma_start(out=wg[:], in_=wgD[m])
                    wu = wgu_pool.tile([P, KH * P], BF16, name="wu")
                    nc.scalar.dma_start(out=wu[:], in_=wuD[m])
                    for (c0, cw) in CS:
                        o = KH * c0
                        pg = psg.tile([P, 512], F32, name="pg")[:, :cw]
                        pu = psu.tile([P, 512], F32, name="pu")[:, :cw]
                        for k in range(KH):
                            nc.tensor.matmul(
                                out=pg[:], lhsT=wg[:, k * P:(k + 1) * P],
                                rhs=xg[:, o + k * cw:o + (k + 1) * cw],
                                start=(k == 0), stop=(k == KH - 1))
                        for k in range(KH):
                            nc.tensor.matmul(
                                out=pu[:], lhsT=wu[:, k * P:(k + 1) * P],
                                rhs=xg[:, o + k * cw:o + (k + 1) * cw],
                                start=(k == 0), stop=(k == KH - 1))
                        sg = tmp_pool.tile([P, 512], F32, name="sg")[:, :cw]
                        nc.scalar.activation(out=sg[:], in_=pg[:], func=ACTF.Silu,
                                             bias=0.0, scale=1.0)
                        nc.vector.tensor_tensor(
                            out=actT[:, m, c0:c0 + cw], in0=sg[:], in1=pu[:],
                            op=ALU.mult)

                # down projection; routing weight applied at eviction; one
                # batched output DMA per 128-token chunk
                for cc in range(CC):
                    rows = min(P, C - cc * P)
                    ev = ev_pool.tile([P, HS * 512], F32, name="ev")
                    for hs in range(HS):
                        pd = psd.tile([P, 512], F32, name="pd")
                        for k in range(KF):
                            wo = (k * HS + hs) * 512
                            nc.tensor.matmul(
                                out=pd[:rows, :],
                                lhsT=actT[:, k, cc * P:cc * P + rows],
                                rhs=wd[:, wo:wo + 512],
                                start=(k == 0), stop=(k == KF - 1))
                        nc.vector.tensor_scalar(
                            out=ev[:rows, hs * 512:(hs + 1) * 512],
                            in0=pd[:rows, :],
                            scalar1=wv[:rows, cc:cc + 1], scalar2=None,
                            op0=ALU.mult)
                    evq = nc.sync if cc % 2 == 0 else nc.scalar
                    evq.dma_start(out=outD[cc, :rows, :],
                                  in_=ev[:rows, :])
    nc.compile()
    return nc


# ---------------------------------------------------------------------------
# host routing (exact fp32 replication of the reference)
# ---------------------------------------------------------------------------
def _host_routing(x, gumbel_u, W1, b1, W2, b2, gate_w, U, alpha):
    h1 = x @ W1.T + b1
    h1 *= 1.0 / (1.0 + np.exp(-h1))                       # silu
    zl = h1 @ W2.T + b2
    g = -np.log(-np.log(gumbel_u + EPS) + EPS)
    s = (zl + g) / TAU
    s -= s.max(-1, keepdims=True)
    es = np.exp(s)
    soft = es / es.sum(-1, keepdims=True)
    hard = np.zeros_like(soft)
    hard[np.arange(T), soft.argmax(-1)] = 1.0
    z = (hard + soft) - soft                              # straight-through
    rl = x @ gate_w.T + np.float32(alpha) * (z @ U)
    rl -= rl.max(-1, keepdims=True)
    er = np.exp(rl)
    rw = er / er.sum(-1, keepdims=True)
    order = np.argsort(-rw, axis=1, kind="stable")[:, :TOP_K]
    topw = np.take_along_axis(rw, order, axis=1)
    return order, topw


def kernel(hidden_states, gumbel_u, W1, b1, W2, b2, gate_w, U, alpha, Wg, Wu, Wd):
    import time as _time

    t_start = _time.time()
    x = np.ascontiguousarray(np.asarray(hidden_states, np.float32).reshape(T, H))

    # ---- routing on host ----
    t0 = _time.time()
    order, topw = _host_routing(
        x, np.asarray(gumbel_u, np.float32),
        np.asarray(W1, np.float32), np.asarray(b1, np.float32),
        np.asarray(W2, np.float32), np.asarray(b2, np.float32),
        np.asarray(gate_w, np.float32), np.asarray(U, np.float32), alpha)
    idxs = [None] * E
    wvals = [None] * E
    tok = np.arange(T)
    for e in range(E):
        rows, cols = np.nonzero(order == e)
        idxs[e] = rows
        wvals[e] = topw[rows, cols].astype(np.float32)
    _timings["routing"] = _time.time() - t0

    # ---- pack pieces into 8 cores x nslots ----
    t0 = _time.time()
    pieces = []
    for e in range(E):
        c = len(idxs[e])
        nparts = max(1, math.ceil(c / CAP))
        base, rem = divmod(c, nparts)
        off = 0
        for i in range(nparts):
            ln = base + (1 if i < rem else 0)
            pieces.append((e, off, ln))
            off += ln

    def cost(ln):
        return 256 * ln + 16384 * math.ceil(ln / P)

    pieces.sort(key=lambda p: -p[2])
    loads = [0] * N_CORES
    assign = [[] for _ in range(N_CORES)]
    for pc in pieces:
        c = min(range(N_CORES), key=lambda i: loads[i])
        assign[c].append(pc)
        loads[c] += cost(pc[2])
    nslots = max(len(a) for a in assign)
    for a in assign:
        a.sort(key=lambda p: -p[2])
        while len(a) < nslots:
            a.append((0, 0, 0))                            # dummy slot
    sizes = [max(P, max(assign[c][i][2] for c in range(N_CORES)))
             for i in range(nslots)]

    # ---- weight/activation prep (bf16, transposed+interleaved) ----
    xT = np.ascontiguousarray(
        x.reshape(T, KH, P).transpose(2, 1, 0).astype(BF))   # [128, 16, T]
    WgB = np.asarray(Wg, np.float32).astype(BF)
    WuB = np.asarray(Wu, np.float32).astype(BF)
    WdB = np.asarray(Wd, np.float32).astype(BF)
    # wgt[e,m,p,k,j] = Wg[e, m*128+j, k*128+p]
    WgT = np.ascontiguousarray(
        WgB.reshape(E, MF, P, KH, P).transpose(0, 1, 4, 3, 2))
    WuT = np.ascontiguousarray(
        WuB.reshape(E, MF, P, KH, P).transpose(0, 1, 4, 3, 2))
    # wdt[e,p,k,hs,j] = Wd[e, hs*512+j, k*128+p]
    WdT = np.ascontiguousarray(
        WdB.reshape(E, HS, 512, KF, P).transpose(0, 4, 3, 1, 2))

    in_maps = []
    for c in range(N_CORES):
        m = {}
        for si in range(nslots):
            e, off, ln = assign[c][si]
            Csz = sizes[si]
            CC = (Csz + P - 1) // P
            xg3 = np.zeros((P, KH, Csz), BF)
            wvp = np.zeros((CC * P,), np.float32)
            if ln > 0:
                sel = idxs[e][off:off + ln]
                xg3[:, :, :ln] = xT[:, :, sel]
                wvp[:ln] = wvals[e][off:off + ln]
            # slice-major flat pack (must match kernel's per-slice offsets)
            xg = np.concatenate(
                [np.ascontiguousarray(xg3[:, :, c0:c0 + cw]).reshape(P, KH * cw)
                 for (c0, cw) in _slices(Csz)], axis=1)
            m[f"xg{si}"] = xg
            m[f"wg{si}"] = WgT[e].reshape(MF, P, KH * P)
            m[f"wu{si}"] = WuT[e].reshape(MF, P, KH * P)
            m[f"wd{si}"] = WdT[e].reshape(P, KF * HS * 512)
            m[f"wv{si}"] = np.ascontiguousarray(wvp.reshape(CC, P).T)
        in_maps.append(m)
    _timings["dispatch"] = _time.time() - t0

    t0 = _time.time()
    key = tuple(sizes)
    nc2 = _build_cache.get(key)
    if nc2 is None:
        nc2 = build_k2(sizes)
        _build_cache[key] = nc2
    _timings["k2_build"] = _time.time() - t0

    t0 = _time.time()
    res2 = run_bass_kernel_spmd(nc2, in_maps, list(range(N_CORES)), trace=TRACE)
    _timings["k2_run"] = _time.time() - t0
    if TRACE:
        _timings["k2_hw_ns"] = res2.exec_time_ns

    # ---- host combine (unshard) ----
    t0 = _time.time()
    y = np.zeros((T, H), np.float32)
    for c in range(N_CORES):
        for si in range(nslots):
            e, off, ln = assign[c][si]
            if ln == 0:
                continue
            oc = res2.results[c][f"out{si}"]             # [CC, 128, 2048]
            y[idxs[e][off:off + ln]] += oc.reshape(-1, H)[:ln]
    _timings["combine"] = _time.time() - t0
    _timings["total"] = _time.time() - t_start
    return y.reshape(B, S, H)


# revision 8
# speedup vs baseline: 1.0529x; 1.0095x over previous
"""Trainium2 Bass kernel for CrossLayerSharedZOlmoeSparseMoeBlock.

Strategy (expert-parallel, 8 cores):
  host: full routing math in fp32 numpy (predictor MLP + gumbel argmax +
        router softmax + top-8-of-16) -> comb [T, E]; per-expert token
        index lists; experts paired best-with-worst by load and assigned
        2 slots/core (slot sizes are compile-time constants = max over
        cores); token buffers gathered/compacted per slot in bf16.
  device (one kernel launch): per core, per slot: gate/up/down matmuls
        in bf16 (fp32 PSUM accumulate), silu*up fused at PSUM eviction,
        routing weight applied on-chip at down-proj eviction. Exact token
        counts (no 128-padding of the moving dim).
  host: scatter-add compact fp32 outputs into y.

bf16 matmuls run at 1 cycle/row on the PE (vs ~1.8 for f32r's
fp32_mode=HIGH lowering) and halve weight/activation DMA traffic.
Aggregate rel err ~4e-3 (tolerance 2e-2).
"""
import contextlib
import ctypes
import math
import os
import sys
import types

import ml_dtypes
import numpy as np

sys.path.insert(0, "/opt/trn_rl_repo")

# ---------------------------------------------------------------------------
# NTFF profile hook shim (antenv.axon_hooks is absent in this image; bass's
# trace=True path imports it). Lets us read HW exec time via neuron profile.
# ---------------------------------------------------------------------------
_SO_PATH = "/opt/axon/libaxon_pjrt.so"


def _ntff_profile_via_ctypes(so_path):
    try:
        lib = ctypes.CDLL(so_path)
    except OSError:
        return None
    if not hasattr(lib, "axon_start_nrt_profile"):
        return None
    lib.axon_start_nrt_profile.argtypes = [ctypes.POINTER(ctypes.c_int64), ctypes.c_size_t]
    lib.axon_start_nrt_profile.restype = ctypes.c_int64
    lib.axon_stop_nrt_profile.argtypes = [ctypes.c_char_p]
    lib.axon_stop_nrt_profile.restype = ctypes.c_int64

    @contextlib.contextmanager
    def _hook(output_dir, device_ids):
        import jax

        jax.devices()
        if device_ids:
            ids = (ctypes.c_int64 * len(device_ids))(*device_ids)
            rc = lib.axon_start_nrt_profile(ids, len(device_ids))
        else:
            rc = lib.axon_start_nrt_profile(None, 0)
        if rc != 0:
            raise RuntimeError(f"axon_start_nrt_profile rc={rc}")
        try:
            yield
        finally:
            n = lib.axon_stop_nrt_profile(str(output_dir).encode())
            print(f"ntff profile: {n} file(s) -> {output_dir}", file=sys.stderr)

    return _hook


def _install_hook():
    if "antenv.axon_hooks" in sys.modules:
        return
    mod = types.ModuleType("antenv.axon_hooks")
    _h = [_ntff_profile_via_ctypes(_SO_PATH)]
    mod.get_axon_ntff_profile_hook = lambda: _h[0]
    mod.set_axon_ntff_profile_hook = lambda h: _h.__setitem__(0, h)
    sys.modules["antenv.axon_hooks"] = mod
    try:
        import antenv

        antenv.axon_hooks = mod
    except ImportError:
        pass


_install_hook()

import concourse.mybir as mybir  # noqa: E402
import concourse.tile as tile  # noqa: E402
from concourse import bacc  # noqa: E402
from concourse.bass_utils import run_bass_kernel_spmd  # noqa: E402

F32 = mybir.dt.float32
BF16 = mybir.dt.bfloat16
ALU = mybir.AluOpType
ACTF = mybir.ActivationFunctionType

# problem shapes (hardcoded per contest rules)
B, S, H = 1, 2048, 2048
T = B * S
E, F = 16, 1024
Z, M = 8, 512
TOP_K = 8
EPS = 1e-10
TAU = 1.0
N_CORES = 8
P = 128
KH = H // P          # 16 contraction chunks over H
MF = F // P          # 8 F tiles for gate/up
KF = F // P          # 8 contraction chunks over F
HS = H // 512        # 4 moving slices of 512 for down-proj
CAP = 1536           # max tokens per slot (SBUF budget guard)

TRACE = bool(int(os.environ.get("BASSMOE_TRACE", "0")))
BF = ml_dtypes.bfloat16

_timings = {}
_build_cache = {}


def _slices(C):
    # progressive first slices so compute can start before the full token
    # buffer lands
    out, off = [], 0
    for w in (128, 256):
        if off >= C:
            return out
        cw = min(w, C - off)
        out.append((off, cw))
        off += cw
    while off < C:
        cw = min(512, C - off)
        out.append((off, cw))
        off += cw
    return out


# ---------------------------------------------------------------------------
# K2: expert kernel. sizes = per-slot token counts (compile-time).
# ---------------------------------------------------------------------------
def build_k2(sizes):
    nc = bacc.Bacc(None, target_bir_lowering=False)
    ins, outs = [], []
    for s, C in enumerate(sizes):
        CC = (C + P - 1) // P
        ins.append((
            nc.dram_tensor(f"xg{s}", [P, KH * C], BF16, kind="ExternalInput"),
            nc.dram_tensor(f"wg{s}", [MF, P, KH * P], BF16, kind="ExternalInput"),
            nc.dram_tensor(f"wu{s}", [MF, P, KH * P], BF16, kind="ExternalInput"),
            nc.dram_tensor(f"wd{s}", [P, KF * HS * 512], BF16,
                           kind="ExternalInput"),
            nc.dram_tensor(f"wv{s}", [P, CC], F32, kind="ExternalInput"),
        ))
        outs.append(nc.dram_tensor(f"out{s}", [CC, P, HS * 512], BF16,
                                   kind="ExternalOutput"))

    with tile.TileContext(nc) as tc:
        with tc.tile_pool(name="xg", bufs=1) as xg_pool, \
             tc.tile_pool(name="act", bufs=2) as act_pool, \
             tc.tile_pool(name="wgu", bufs=2) as wgu_pool, \
             tc.tile_pool(name="wd", bufs=1) as wd_pool, \
             tc.tile_pool(name="wvp", bufs=2) as wv_pool, \
             tc.tile_pool(name="tmp", bufs=3) as tmp_pool, \
             tc.tile_pool(name="ev", bufs=3) as ev_pool, \
             tc.tile_pool(name="psg", bufs=2, space="PSUM") as psg, \
             tc.tile_pool(name="psu", bufs=2, space="PSUM") as psu, \
             tc.tile_pool(name="psd", bufs=2, space="PSUM") as psd:
            # PE warmup (HAM unthrottle) while the first DMAs land. The warm
            # tile is the wd-pool buffer: wd prefetches then wait for warmup
            # reads instead of racing the critical xg/wg DMAs at t=0.
            warm = wd_pool.tile([P, KF * HS * 512], BF16, name="wd")
            nc.vector.memset(warm[:, :512], 0.0)
            for i in range(8):
                wps = (psg if i % 2 == 0 else psu).tile(
                    [P, 512], F32, name=("pg" if i % 2 == 0 else "pu"))
                nc.tensor.matmul(out=wps[:], lhsT=warm[:, :P],
                                 rhs=warm[:, :512], start=True, stop=True)

            for s, C in enumerate(sizes):
                xgD, wgD, wuD, wdD, wvD = ins[s]
                outD = outs[s]
                CC = (C + P - 1) // P
                CS = _slices(C)

                # token buffer, slice-major: each slice is one contiguous
                # per-partition run (full DMA line rate), alternating queues;
                # the first (small) slices land fast so compute starts early
                xg = xg_pool.tile([P, KH * C], BF16, name="xg")
                for si, (c0, cw) in enumerate(CS):
                    o = KH * c0
                    q = nc.sync if si % 2 == 0 else nc.gpsimd
                    q.dma_start(out=xg[:, o:o + KH * cw],
                                in_=xgD[:, o:o + KH * cw])
                wv = wv_pool.tile([P, CC], F32, name="wv")
                nc.sync.dma_start(out=wv[:], in_=wvD[:])

                actT = act_pool.tile([P, KF, C], BF16, name="actT")
                wd = None
                if s > 0:
                    # buffer reuse (bufs=1) already delays this DMA until the
                    # previous slot's down-proj has consumed its weights
                    wd = wd_pool.tile([P, KF * HS * 512], BF16, name="wd")
                    nc.scalar.dma_start(out=wd[:], in_=wdD[:])
                for m in range(MF):
                    if m == 1 and s == 0:
                        # guard read of the warm buffer, gated on m=0 output:
                        # the wd prefetch then cannot start before real
                        # compute is underway, keeping HBM free for the
                        # critical xg/wg loads at t=0
                        scr = tmp_pool.tile([P, 1], F32, name="scr")
                        nc.vector.tensor_tensor(
                            out=scr[:], in0=warm[:, :1],
                            in1=actT[:, 0, 0:1], op=ALU.mult)
                        wd = wd_pool.tile([P, KF * HS * 512], BF16, name="wd")
                        nc.scalar.dma_start(out=wd[:], in_=wdD[:])
                    wg = wgu_pool.tile([P, KH * P], BF16, name="wg")
                    nc.scalar.dma_start(out=wg[:], in_=wgD[m])
                    wu = wgu_pool.tile([P, KH * P], BF16, name="wu")
                    nc.scalar.dma_start(out=wu[:], in_=wuD[m])
                    for (c0, cw) in CS:
                        o = KH * c0
                        pg = psg.tile([P, 512], F32, name="pg")[:, :cw]
                        pu = psu.tile([P, 512], F32, name="pu")[:, :cw]
                        for k in range(KH):
                            nc.tensor.matmul(
                                out=pg[:], lhsT=wg[:, k * P:(k + 1) * P],
                                rhs=xg[:, o + k * cw:o + (k + 1) * cw],
                                start=(k == 0), stop=(k == KH - 1))
                        for k in range(KH):
                            nc.tensor.matmul(
                                out=pu[:], lhsT=wu[:, k * P:(k + 1) * P],
                                rhs=xg[:, o + k * cw:o + (k + 1) * cw],
                                start=(k == 0), stop=(k == KH - 1))
                        sg = tmp_pool.tile([P, 512], F32, name="sg")[:, :cw]
                        nc.scalar.activation(out=sg[:], in_=pg[:], func=ACTF.Silu,
                                             bias=0.0, scale=1.0)
                        nc.vector.tensor_tensor(
                            out=actT[:, m, c0:c0 + cw], in0=sg[:], in1=pu[:],
                            op=ALU.mult)

                # down projection; routing weight applied at eviction; one
                # batched output DMA per 128-token chunk
                for cc in range(CC):
                    rows = min(P, C - cc * P)
                    ev = ev_pool.tile([P, HS * 512], BF16, name="ev")
                    for hs in range(HS):
                        pd = psd.tile([P, 512], F32, name="pd")
                        for k in range(KF):
                            wo = (k * HS + hs) * 512
                            nc.tensor.matmul(
                                out=pd[:rows, :],
                                lhsT=actT[:, k, cc * P:cc * P + rows],
                                rhs=wd[:, wo:wo + 512],
                                start=(k == 0), stop=(k == KF - 1))
                        nc.vector.tensor_scalar(
                            out=ev[:rows, hs * 512:(hs + 1) * 512],
                            in0=pd[:rows, :],
                            scalar1=wv[:rows, cc:cc + 1], scalar2=None,
                            op0=ALU.mult)
                    evq = nc.sync if cc % 2 == 0 else nc.scalar
                    evq.dma_start(out=outD[cc, :rows, :],
                                  in_=ev[:rows, :])
    nc.compile()
    return nc


# ---------------------------------------------------------------------------
# host routing (exact fp32 replication of the reference)
# ---------------------------------------------------------------------------
def _host_routing(x, gumbel_u, W1, b1, W2, b2, gate_w, U, alpha):
    h1 = x @ W1.T + b1
    h1 *= 1.0 / (1.0 + np.exp(-h1))                       # silu
    zl = h1 @ W2.T + b2
    g = -np.log(-np.log(gumbel_u + EPS) + EPS)
    s = (zl + g) / TAU
    s -= s.max(-1, keepdims=True)
    es = np.exp(s)
    soft = es / es.sum(-1, keepdims=True)
    hard = np.zeros_like(soft)
    hard[np.arange(T), soft.argmax(-1)] = 1.0
    z = (hard + soft) - soft                              # straight-through
    rl = x @ gate_w.T + np.float32(alpha) * (z @ U)
    rl -= rl.max(-1, keepdims=True)
    er = np.exp(rl)
    rw = er / er.sum(-1, keepdims=True)
    order = np.argsort(-rw, axis=1, kind="stable")[:, :TOP_K]
    topw = np.take_along_axis(rw, order, axis=1)
    return order, topw


def kernel(hidden_states, gumbel_u, W1, b1, W2, b2, gate_w, U, alpha, Wg, Wu, Wd):
    import time as _time

    t_start = _time.time()
    x = np.ascontiguousarray(np.asarray(hidden_states, np.float32).reshape(T, H))

    # ---- routing on host ----
    t0 = _time.time()
    order, topw = _host_routing(
        x, np.asarray(gumbel_u, np.float32),
        np.asarray(W1, np.float32), np.asarray(b1, np.float32),
        np.asarray(W2, np.float32), np.asarray(b2, np.float32),
        np.asarray(gate_w, np.float32), np.asarray(U, np.float32), alpha)
    idxs = [None] * E
    wvals = [None] * E
    tok = np.arange(T)
    for e in range(E):
        rows, cols = np.nonzero(order == e)
        idxs[e] = rows
        wvals[e] = topw[rows, cols].astype(np.float32)
    _timings["routing"] = _time.time() - t0

    # ---- pack pieces into 8 cores x nslots ----
    t0 = _time.time()
    pieces = []
    for e in range(E):
        c = len(idxs[e])
        nparts = max(1, math.ceil(c / CAP))
        base, rem = divmod(c, nparts)
        off = 0
        for i in range(nparts):
            ln = base + (1 if i < rem else 0)
            pieces.append((e, off, ln))
            off += ln

    def cost(ln):
        return 256 * ln + 16384 * math.ceil(ln / P)

    pieces.sort(key=lambda p: -p[2])
    loads = [0] * N_CORES
    assign = [[] for _ in range(N_CORES)]
    for pc in pieces:
        c = min(range(N_CORES), key=lambda i: loads[i])
        assign[c].append(pc)
        loads[c] += cost(pc[2])
    nslots = max(len(a) for a in assign)
    for a in assign:
        a.sort(key=lambda p: -p[2])
        while len(a) < nslots:
            a.append((0, 0, 0))                            # dummy slot
    sizes = [max(P, max(assign[c][i][2] for c in range(N_CORES)))
             for i in range(nslots)]

    # ---- weight/activation prep (bf16, transposed+interleaved) ----
    xT = np.ascontiguousarray(
        x.reshape(T, KH, P).transpose(2, 1, 0).astype(BF))   # [128, 16, T]
    WgB = np.asarray(Wg, np.float32).astype(BF)
    WuB = np.asarray(Wu, np.float32).astype(BF)
    WdB = np.asarray(Wd, np.float32).astype(BF)
    # wgt[e,m,p,k,j] = Wg[e, m*128+j, k*128+p]
    WgT = np.ascontiguousarray(
        WgB.reshape(E, MF, P, KH, P).transpose(0, 1, 4, 3, 2))
    WuT = np.ascontiguousarray(
        WuB.reshape(E, MF, P, KH, P).transpose(0, 1, 4, 3, 2))
    # wdt[e,p,k,hs,j] = Wd[e, hs*512+j, k*128+p]
    WdT = np.ascontiguousarray(
        WdB.reshape(E, HS, 512, KF, P).transpose(0, 4, 3, 1, 2))

    in_maps = []
    for c in range(N_CORES):
        m = {}
        for si in range(nslots):
            e, off, ln = assign[c][si]
            Csz = sizes[si]
            CC = (Csz + P - 1) // P
            xg3 = np.zeros((P, KH, Csz), BF)
            wvp = np.zeros((CC * P,), np.float32)
            if ln > 0:
                sel = idxs[e][off:off + ln]
                xg3[:, :, :ln] = xT[:, :, sel]
                wvp[:ln] = wvals[e][off:off + ln]
            # slice-major flat pack (must match kernel's per-slice offsets)
            xg = np.concatenate(
                [np.ascontiguousarray(xg3[:, :, c0:c0 + cw]).reshape(P, KH * cw)
                 for (c0, cw) in _slices(Csz)], axis=1)
            m[f"xg{si}"] = xg
            m[f"wg{si}"] = WgT[e].reshape(MF, P, KH * P)
            m[f"wu{si}"] = WuT[e].reshape(MF, P, KH * P)
            m[f"wd{si}"] = WdT[e].reshape(P, KF * HS * 512)
            m[f"wv{si}"] = np.ascontiguousarray(wvp.reshape(CC, P).T)
        in_maps.append(m)
    _timings["dispatch"] = _time.time() - t0

    t0 = _time.time()
    key = tuple(sizes)
    nc2 = _build_cache.get(key)
    if nc2 is None:
        nc2 = build_k2(sizes)
        _build_cache[key] = nc2
    _timings["k2_build"] = _time.time() - t0

    t0 = _time.time()
    res2 = run_bass_kernel_spmd(nc2, in_maps, list(range(N_CORES)), trace=TRACE)
    _timings["k2_run"] = _time.time() - t0
    if TRACE:
        _timings["k2_hw_ns"] = res2.exec_time_ns

    # ---- host combine (unshard) ----
    t0 = _time.time()
    y = np.zeros((T, H), np.float32)
    for c in range(N_CORES):
        for si in range(nslots):
            e, off, ln = assign[c][si]
            if ln == 0:
                continue
            oc = res2.results[c][f"out{si}"]             # [CC, 128, 2048] bf16
            y[idxs[e][off:off + ln]] += oc.reshape(-1, H)[:ln].astype(np.float32)
    _timings["combine"] = _time.time() - t0
    _timings["total"] = _time.time() - t_start
    return y.reshape(B, S, H)


# revision 9
# speedup vs baseline: 1.0630x; 1.0096x over previous
"""Trainium2 Bass kernel for CrossLayerSharedZOlmoeSparseMoeBlock.

Strategy (expert-parallel, 8 cores):
  host: full routing math in fp32 numpy (predictor MLP + gumbel argmax +
        router softmax + top-8-of-16) -> comb [T, E]; per-expert token
        index lists; experts paired best-with-worst by load and assigned
        2 slots/core (slot sizes are compile-time constants = max over
        cores); token buffers gathered/compacted per slot in bf16.
  device (one kernel launch): per core, per slot: gate/up/down matmuls
        in bf16 (fp32 PSUM accumulate), silu*up fused at PSUM eviction,
        routing weight applied on-chip at down-proj eviction. Exact token
        counts (no 128-padding of the moving dim).
  host: scatter-add compact fp32 outputs into y.

bf16 matmuls run at 1 cycle/row on the PE (vs ~1.8 for f32r's
fp32_mode=HIGH lowering) and halve weight/activation DMA traffic.
Aggregate rel err ~4e-3 (tolerance 2e-2).
"""
import contextlib
import ctypes
import math
import os
import sys
import types

import ml_dtypes
import numpy as np

sys.path.insert(0, "/opt/trn_rl_repo")

# ---------------------------------------------------------------------------
# NTFF profile hook shim (antenv.axon_hooks is absent in this image; bass's
# trace=True path imports it). Lets us read HW exec time via neuron profile.
# ---------------------------------------------------------------------------
_SO_PATH = "/opt/axon/libaxon_pjrt.so"


def _ntff_profile_via_ctypes(so_path):
    try:
        lib = ctypes.CDLL(so_path)
    except OSError:
        return None
    if not hasattr(lib, "axon_start_nrt_profile"):
        return None
    lib.axon_start_nrt_profile.argtypes = [ctypes.POINTER(ctypes.c_int64), ctypes.c_size_t]
    lib.axon_start_nrt_profile.restype = ctypes.c_int64
    lib.axon_stop_nrt_profile.argtypes = [ctypes.c_char_p]
    lib.axon_stop_nrt_profile.restype = ctypes.c_int64

    @contextlib.contextmanager
    def _hook(output_dir, device_ids):
        import jax

        jax.devices()
        if device_ids:
            ids = (ctypes.c_int64 * len(device_ids))(*device_ids)
            rc = lib.axon_start_nrt_profile(ids, len(device_ids))
        else:
            rc = lib.axon_start_nrt_profile(None, 0)
        if rc != 0:
            raise RuntimeError(f"axon_start_nrt_profile rc={rc}")
        try:
            yield
        finally:
            n = lib.axon_stop_nrt_profile(str(output_dir).encode())
            print(f"ntff profile: {n} file(s) -> {output_dir}", file=sys.stderr)

    return _hook


def _install_hook():
    if "antenv.axon_hooks" in sys.modules:
        return
    mod = types.ModuleType("antenv.axon_hooks")
    _h = [_ntff_profile_via_ctypes(_SO_PATH)]
    mod.get_axon_ntff_profile_hook = lambda: _h[0]
    mod.set_axon_ntff_profile_hook = lambda h: _h.__setitem__(0, h)
    sys.modules["antenv.axon_hooks"] = mod
    try:
        import antenv

        antenv.axon_hooks = mod
    except ImportError:
        pass


_install_hook()

import concourse.mybir as mybir  # noqa: E402
import concourse.tile as tile  # noqa: E402
from concourse import bacc  # noqa: E402
from concourse.bass_utils import run_bass_kernel_spmd  # noqa: E402

F32 = mybir.dt.float32
BF16 = mybir.dt.bfloat16
ALU = mybir.AluOpType
ACTF = mybir.ActivationFunctionType

# problem shapes (hardcoded per contest rules)
B, S, H = 1, 2048, 2048
T = B * S
E, F = 16, 1024
Z, M = 8, 512
TOP_K = 8
EPS = 1e-10
TAU = 1.0
N_CORES = 8
P = 128
KH = H // P          # 16 contraction chunks over H
MF = F // P          # 8 F tiles for gate/up
KF = F // P          # 8 contraction chunks over F
HS = H // 512        # 4 moving slices of 512 for down-proj
CAP = 1536           # max tokens per slot (SBUF budget guard)

TRACE = bool(int(os.environ.get("BASSMOE_TRACE", "0")))
BF = ml_dtypes.bfloat16

_timings = {}
_build_cache = {}


def _slices(C):
    # progressive first slices so compute can start before the full token
    # buffer lands
    out, off = [], 0
    for w in (128, 256):
        if off >= C:
            return out
        cw = min(w, C - off)
        out.append((off, cw))
        off += cw
    while off < C:
        cw = min(512, C - off)
        out.append((off, cw))
        off += cw
    return out


# ---------------------------------------------------------------------------
# K2: expert kernel. sizes = per-slot token counts (compile-time).
# ---------------------------------------------------------------------------
def build_k2(sizes):
    nc = bacc.Bacc(None, target_bir_lowering=False)
    ins, outs = [], []
    for s, C in enumerate(sizes):
        CC = (C + P - 1) // P
        ins.append((
            nc.dram_tensor(f"xg{s}", [P, KH * C], BF16, kind="ExternalInput"),
            nc.dram_tensor(f"wg{s}", [MF, P, KH * P], BF16, kind="ExternalInput"),
            nc.dram_tensor(f"wu{s}", [MF, P, KH * P], BF16, kind="ExternalInput"),
            nc.dram_tensor(f"wd{s}", [P, KF * HS * 512], BF16,
                           kind="ExternalInput"),
            nc.dram_tensor(f"wv{s}", [P, CC], F32, kind="ExternalInput"),
        ))
        outs.append(nc.dram_tensor(f"out{s}", [CC, P, HS * 512], BF16,
                                   kind="ExternalOutput"))

    with tile.TileContext(nc) as tc:
        with tc.tile_pool(name="xg", bufs=1) as xg_pool, \
             tc.tile_pool(name="act", bufs=2) as act_pool, \
             tc.tile_pool(name="wgu", bufs=2) as wgu_pool, \
             tc.tile_pool(name="wd", bufs=1) as wd_pool, \
             tc.tile_pool(name="wvp", bufs=2) as wv_pool, \
             tc.tile_pool(name="tmp", bufs=3) as tmp_pool, \
             tc.tile_pool(name="ev", bufs=3) as ev_pool, \
             tc.tile_pool(name="psg", bufs=2, space="PSUM") as psg, \
             tc.tile_pool(name="psu", bufs=2, space="PSUM") as psu, \
             tc.tile_pool(name="psd", bufs=2, space="PSUM") as psd:
            # PE warmup (HAM unthrottle) while the first DMAs land. The warm
            # tile is the wd-pool buffer, and dmy_g/dmy_u occupy one wgu slot
            # each: guard reads of these buffers (gated on m=0 output) keep
            # the wd/wg1/wu1 prefetches from racing the critical first xg/wg
            # loads at t=0.
            warm = wd_pool.tile([P, KF * HS * 512], BF16, name="wd")
            nc.vector.memset(warm[:, :512], 0.0)
            dmy_g = wgu_pool.tile([P, KH * P], BF16, name="wg")
            nc.gpsimd.memset(dmy_g[:, :1], 0.0)
            dmy_u = wgu_pool.tile([P, KH * P], BF16, name="wu")
            nc.gpsimd.memset(dmy_u[:, :1], 0.0)
            for i in range(8):
                wps = (psg if i % 2 == 0 else psu).tile(
                    [P, 512], F32, name=("pg" if i % 2 == 0 else "pu"))
                nc.tensor.matmul(out=wps[:], lhsT=warm[:, :P],
                                 rhs=warm[:, :512], start=True, stop=True)

            for s, C in enumerate(sizes):
                xgD, wgD, wuD, wdD, wvD = ins[s]
                outD = outs[s]
                CC = (C + P - 1) // P
                CS = _slices(C)

                # token buffer, slice-major: each slice is one contiguous
                # per-partition run (full DMA line rate). Slot 0 is the
                # startup critical path: first weight tiles head the two
                # HWDGE queues, then token slices stream k-half-split across
                # both in consumption order. Later slots load while the
                # previous slot computes, so ordering there is relaxed.
                xg = xg_pool.tile([P, KH * C], BF16, name="xg")
                wg0 = wu0 = None
                if s == 0:
                    wg0 = wgu_pool.tile([P, KH * P], BF16, name="wg")
                    nc.sync.dma_start(out=wg0[:], in_=wgD[0])
                    wu0 = wgu_pool.tile([P, KH * P], BF16, name="wu")
                    nc.scalar.dma_start(out=wu0[:], in_=wuD[0])
                    for (c0, cw) in CS:
                        o, half = KH * c0, KH // 2 * cw
                        nc.sync.dma_start(out=xg[:, o:o + half],
                                          in_=xgD[:, o:o + half])
                        nc.scalar.dma_start(out=xg[:, o + half:o + KH * cw],
                                            in_=xgD[:, o + half:o + KH * cw])
                else:
                    for si, (c0, cw) in enumerate(CS):
                        o = KH * c0
                        q = nc.sync if si % 2 == 0 else nc.gpsimd
                        q.dma_start(out=xg[:, o:o + KH * cw],
                                    in_=xgD[:, o:o + KH * cw])
                wv = wv_pool.tile([P, CC], F32, name="wv")
                nc.sync.dma_start(out=wv[:], in_=wvD[:])

                actT = act_pool.tile([P, KF, C], BF16, name="actT")
                wd = None
                if s > 0:
                    # buffer reuse (bufs=1) already delays this DMA until the
                    # previous slot's down-proj has consumed its weights
                    wd = wd_pool.tile([P, KF * HS * 512], BF16, name="wd")
                    nc.scalar.dma_start(out=wd[:], in_=wdD[:])
                for m in range(MF):
                    if m == 1 and s == 0:
                        # guard reads of the dummy/warm buffers, gated on m=0
                        # output: wg1/wu1/wd prefetches (next users of those
                        # pool slots) then cannot start before real compute
                        # is underway, keeping HBM free for the critical
                        # first xg/wg loads
                        for gsrc in (dmy_g, dmy_u, warm):
                            scr = tmp_pool.tile([P, 1], F32, name="scr")
                            nc.vector.tensor_tensor(
                                out=scr[:], in0=gsrc[:, :1],
                                in1=actT[:, 0, 0:1], op=ALU.mult)
                        wd = wd_pool.tile([P, KF * HS * 512], BF16, name="wd")
                        nc.scalar.dma_start(out=wd[:], in_=wdD[:])
                    if m == 0 and s == 0:
                        wg, wu = wg0, wu0
                    else:
                        wg = wgu_pool.tile([P, KH * P], BF16, name="wg")
                        nc.scalar.dma_start(out=wg[:], in_=wgD[m])
                        wu = wgu_pool.tile([P, KH * P], BF16, name="wu")
                        nc.scalar.dma_start(out=wu[:], in_=wuD[m])
                    for (c0, cw) in CS:
                        o = KH * c0
                        pg = psg.tile([P, 512], F32, name="pg")[:, :cw]
                        pu = psu.tile([P, 512], F32, name="pu")[:, :cw]
                        for k in range(KH):
                            nc.tensor.matmul(
                                out=pg[:], lhsT=wg[:, k * P:(k + 1) * P],
                                rhs=xg[:, o + k * cw:o + (k + 1) * cw],
                                start=(k == 0), stop=(k == KH - 1))
                        for k in range(KH):
                            nc.tensor.matmul(
                                out=pu[:], lhsT=wu[:, k * P:(k + 1) * P],
                                rhs=xg[:, o + k * cw:o + (k + 1) * cw],
                                start=(k == 0), stop=(k == KH - 1))
                        sg = tmp_pool.tile([P, 512], F32, name="sg")[:, :cw]
                        nc.scalar.activation(out=sg[:], in_=pg[:], func=ACTF.Silu,
                                             bias=0.0, scale=1.0)
                        nc.vector.tensor_tensor(
                            out=actT[:, m, c0:c0 + cw], in0=sg[:], in1=pu[:],
                            op=ALU.mult)

                # down projection; routing weight applied at eviction; one
                # batched output DMA per 128-token chunk
                for cc in range(CC):
                    rows = min(P, C - cc * P)
                    ev = ev_pool.tile([P, HS * 512], BF16, name="ev")
                    for hs in range(HS):
                        pd = psd.tile([P, 512], F32, name="pd")
                        for k in range(KF):
                            wo = (k * HS + hs) * 512
                            nc.tensor.matmul(
                                out=pd[:rows, :],
                                lhsT=actT[:, k, cc * P:cc * P + rows],
                                rhs=wd[:, wo:wo + 512],
                                start=(k == 0), stop=(k == KF - 1))
                        nc.vector.tensor_scalar(
                            out=ev[:rows, hs * 512:(hs + 1) * 512],
                            in0=pd[:rows, :],
                            scalar1=wv[:rows, cc:cc + 1], scalar2=None,
                            op0=ALU.mult)
                    evq = nc.sync if cc % 2 == 0 else nc.scalar
                    evq.dma_start(out=outD[cc, :rows, :],
                                  in_=ev[:rows, :])
    nc.compile()
    return nc


# ---------------------------------------------------------------------------
# host routing (exact fp32 replication of the reference)
# ---------------------------------------------------------------------------
def _host_routing(x, gumbel_u, W1, b1, W2, b2, gate_w, U, alpha):
    h1 = x @ W1.T + b1
    h1 *= 1.0 / (1.0 + np.exp(-h1))                       # silu
    zl = h1 @ W2.T + b2
    g = -np.log(-np.log(gumbel_u + EPS) + EPS)
    s = (zl + g) / TAU
    s -= s.max(-1, keepdims=True)
    es = np.exp(s)
    soft = es / es.sum(-1, keepdims=True)
    hard = np.zeros_like(soft)
    hard[np.arange(T), soft.argmax(-1)] = 1.0
    z = (hard + soft) - soft                              # straight-through
    rl = x @ gate_w.T + np.float32(alpha) * (z @ U)
    rl -= rl.max(-1, keepdims=True)
    er = np.exp(rl)
    rw = er / er.sum(-1, keepdims=True)
    order = np.argsort(-rw, axis=1, kind="stable")[:, :TOP_K]
    topw = np.take_along_axis(rw, order, axis=1)
    return order, topw


def kernel(hidden_states, gumbel_u, W1, b1, W2, b2, gate_w, U, alpha, Wg, Wu, Wd):
    import time as _time

    t_start = _time.time()
    x = np.ascontiguousarray(np.asarray(hidden_states, np.float32).reshape(T, H))

    # ---- routing on host ----
    t0 = _time.time()
    order, topw = _host_routing(
        x, np.asarray(gumbel_u, np.float32),
        np.asarray(W1, np.float32), np.asarray(b1, np.float32),
        np.asarray(W2, np.float32), np.asarray(b2, np.float32),
        np.asarray(gate_w, np.float32), np.asarray(U, np.float32), alpha)
    idxs = [None] * E
    wvals = [None] * E
    tok = np.arange(T)
    for e in range(E):
        rows, cols = np.nonzero(order == e)
        idxs[e] = rows
        wvals[e] = topw[rows, cols].astype(np.float32)
    _timings["routing"] = _time.time() - t0

    # ---- pack pieces into 8 cores x nslots ----
    t0 = _time.time()
    pieces = []
    for e in range(E):
        c = len(idxs[e])
        nparts = max(1, math.ceil(c / CAP))
        base, rem = divmod(c, nparts)
        off = 0
        for i in range(nparts):
            ln = base + (1 if i < rem else 0)
            pieces.append((e, off, ln))
            off += ln

    def cost(ln):
        return 256 * ln + 16384 * math.ceil(ln / P)

    pieces.sort(key=lambda p: -p[2])
    loads = [0] * N_CORES
    assign = [[] for _ in range(N_CORES)]
    for pc in pieces:
        c = min(range(N_CORES), key=lambda i: loads[i])
        assign[c].append(pc)
        loads[c] += cost(pc[2])
    nslots = max(len(a) for a in assign)
    for a in assign:
        a.sort(key=lambda p: -p[2])
        while len(a) < nslots:
            a.append((0, 0, 0))                            # dummy slot
    sizes = [max(P, max(assign[c][i][2] for c in range(N_CORES)))
             for i in range(nslots)]

    # ---- weight/activation prep (bf16, transposed+interleaved) ----
    xT = np.ascontiguousarray(
        x.reshape(T, KH, P).transpose(2, 1, 0).astype(BF))   # [128, 16, T]
    WgB = np.asarray(Wg, np.float32).astype(BF)
    WuB = np.asarray(Wu, np.float32).astype(BF)
    WdB = np.asarray(Wd, np.float32).astype(BF)
    # wgt[e,m,p,k,j] = Wg[e, m*128+j, k*128+p]
    WgT = np.ascontiguousarray(
        WgB.reshape(E, MF, P, KH, P).transpose(0, 1, 4, 3, 2))
    WuT = np.ascontiguousarray(
        WuB.reshape(E, MF, P, KH, P).transpose(0, 1, 4, 3, 2))
    # wdt[e,p,k,hs,j] = Wd[e, hs*512+j, k*128+p]
    WdT = np.ascontiguousarray(
        WdB.reshape(E, HS, 512, KF, P).transpose(0, 4, 3, 1, 2))

    in_maps = []
    for c in range(N_CORES):
        m = {}
        for si in range(nslots):
            e, off, ln = assign[c][si]
            Csz = sizes[si]
            CC = (Csz + P - 1) // P
            xg3 = np.zeros((P, KH, Csz), BF)
            wvp = np.zeros((CC * P,), np.float32)
            if ln > 0:
                sel = idxs[e][off:off + ln]
                xg3[:, :, :ln] = xT[:, :, sel]
                wvp[:ln] = wvals[e][off:off + ln]
            # slice-major flat pack (must match kernel's per-slice offsets)
            xg = np.concatenate(
                [np.ascontiguousarray(xg3[:, :, c0:c0 + cw]).reshape(P, KH * cw)
                 for (c0, cw) in _slices(Csz)], axis=1)
            m[f"xg{si}"] = xg
            m[f"wg{si}"] = WgT[e].reshape(MF, P, KH * P)
            m[f"wu{si}"] = WuT[e].reshape(MF, P, KH * P)
            m[f"wd{si}"] = WdT[e].reshape(P, KF * HS * 512)
            m[f"wv{si}"] = np.ascontiguousarray(wvp.reshape(CC, P).T)
        in_maps.append(m)
    _timings["dispatch"] = _time.time() - t0

    t0 = _time.time()
    key = tuple(sizes)
    nc2 = _build_cache.get(key)
    if nc2 is None:
        nc2 = build_k2(sizes)
        _build_cache[key] = nc2
    _timings["k2_build"] = _time.time() - t0

    t0 = _time.time()
    res2 = run_bass_kernel_spmd(nc2, in_maps, list(range(N_CORES)), trace=TRACE)
    _timings["k2_run"] = _time.time() - t0
    if TRACE:
        _timings["k2_hw_ns"] = res2.exec_time_ns

    # ---- host combine (unshard) ----
    t0 = _time.time()
    y = np.zeros((T, H), np.float32)
    for c in range(N_CORES):
        for si in range(nslots):
            e, off, ln = assign[c][si]
            if ln == 0:
                continue
            oc = res2.results[c][f"out{si}"]             # [CC, 128, 2048] bf16
            y[idxs[e][off:off + ln]] += oc.reshape(-1, H)[:ln].astype(np.float32)
    _timings["combine"] = _time.time() - t0
    _timings["total"] = _time.time() - t_start
    return y.reshape(B, S, H)


# revision 10
# speedup vs baseline: 1.0817x; 1.0175x over previous
"""Trainium2 Bass kernel for CrossLayerSharedZOlmoeSparseMoeBlock.

Strategy (expert-parallel, 8 cores):
  host: full routing math in fp32 numpy (predictor MLP + gumbel argmax +
        router softmax + top-8-of-16) -> comb [T, E]; per-expert token
        index lists; experts paired best-with-worst by load and assigned
        2 slots/core (slot sizes are compile-time constants = max over
        cores); token buffers gathered/compacted per slot in bf16.
  device (one kernel launch): per core, per slot: gate/up/down matmuls
        in bf16 (fp32 PSUM accumulate), silu*up fused at PSUM eviction,
        routing weight applied on-chip at down-proj eviction. Exact token
        counts (no 128-padding of the moving dim).
  host: scatter-add compact fp32 outputs into y.

bf16 matmuls run at 1 cycle/row on the PE (vs ~1.8 for f32r's
fp32_mode=HIGH lowering) and halve weight/activation DMA traffic.
Aggregate rel err ~4e-3 (tolerance 2e-2).
"""
import contextlib
import ctypes
import math
import os
import sys
import types

import ml_dtypes
import numpy as np

sys.path.insert(0, "/opt/trn_rl_repo")

# ---------------------------------------------------------------------------
# NTFF profile hook shim (antenv.axon_hooks is absent in this image; bass's
# trace=True path imports it). Lets us read HW exec time via neuron profile.
# ---------------------------------------------------------------------------
_SO_PATH = "/opt/axon/libaxon_pjrt.so"


def _ntff_profile_via_ctypes(so_path):
    try:
        lib = ctypes.CDLL(so_path)
    except OSError:
        return None
    if not hasattr(lib, "axon_start_nrt_profile"):
        return None
    lib.axon_start_nrt_profile.argtypes = [ctypes.POINTER(ctypes.c_int64), ctypes.c_size_t]
    lib.axon_start_nrt_profile.restype = ctypes.c_int64
    lib.axon_stop_nrt_profile.argtypes = [ctypes.c_char_p]
    lib.axon_stop_nrt_profile.restype = ctypes.c_int64

    @contextlib.contextmanager
    def _hook(output_dir, device_ids):
        import jax

        jax.devices()
        if device_ids:
            ids = (ctypes.c_int64 * len(device_ids))(*device_ids)
            rc = lib.axon_start_nrt_profile(ids, len(device_ids))
        else:
            rc = lib.axon_start_nrt_profile(None, 0)
        if rc != 0:
            raise RuntimeError(f"axon_start_nrt_profile rc={rc}")
        try:
            yield
        finally:
            n = lib.axon_stop_nrt_profile(str(output_dir).encode())
            print(f"ntff profile: {n} file(s) -> {output_dir}", file=sys.stderr)

    return _hook


def _install_hook():
    if "antenv.axon_hooks" in sys.modules:
        return
    mod = types.ModuleType("antenv.axon_hooks")
    _h = [_ntff_profile_via_ctypes(_SO_PATH)]
    mod.get_axon_ntff_profile_hook = lambda: _h[0]
    mod.set_axon_ntff_profile_hook = lambda h: _h.__setitem__(0, h)
    sys.modules["antenv.axon_hooks"] = mod
    try:
        import antenv

        antenv.axon_hooks = mod
    except ImportError:
        pass


_install_hook()

import concourse.mybir as mybir  # noqa: E402
import concourse.tile as tile  # noqa: E402
from concourse import bacc  # noqa: E402
from concourse.bass_utils import run_bass_kernel_spmd  # noqa: E402

F32 = mybir.dt.float32
BF16 = mybir.dt.bfloat16
ALU = mybir.AluOpType
ACTF = mybir.ActivationFunctionType

# problem shapes (hardcoded per contest rules)
B, S, H = 1, 2048, 2048
T = B * S
E, F = 16, 1024
Z, M = 8, 512
TOP_K = 8
EPS = 1e-10
TAU = 1.0
N_CORES = 8
P = 128
KH = H // P          # 16 contraction chunks over H
MF = F // P          # 8 F tiles for gate/up
KF = F // P          # 8 contraction chunks over F
HS = H // 512        # 4 moving slices of 512 for down-proj
CAP = 1536           # max tokens per slot (SBUF budget guard)

TRACE = bool(int(os.environ.get("BASSMOE_TRACE", "0")))
BF = ml_dtypes.bfloat16

_timings = {}
_build_cache = {}


def _slices(C):
    # progressive first slices so compute can start before the full token
    # buffer lands
    out, off = [], 0
    for w in (128, 256):
        if off >= C:
            return out
        cw = min(w, C - off)
        out.append((off, cw))
        off += cw
    while off < C:
        cw = min(512, C - off)
        out.append((off, cw))
        off += cw
    return out


# ---------------------------------------------------------------------------
# K2: expert kernel. sizes = per-slot token counts (compile-time).
# ---------------------------------------------------------------------------
def build_k2(sizes):
    nc = bacc.Bacc(None, target_bir_lowering=False)
    ins, outs = [], []
    for s, C in enumerate(sizes):
        CC = (C + P - 1) // P
        ins.append((
            nc.dram_tensor(f"xg{s}", [P, KH * C], BF16, kind="ExternalInput"),
            nc.dram_tensor(f"wg{s}", [MF, P, KH * P], BF16, kind="ExternalInput"),
            nc.dram_tensor(f"wu{s}", [MF, P, KH * P], BF16, kind="ExternalInput"),
            nc.dram_tensor(f"wd{s}", [P, KF * HS * 512], BF16,
                           kind="ExternalInput"),
            nc.dram_tensor(f"wv{s}", [P, CC], F32, kind="ExternalInput"),
        ))
        outs.append(nc.dram_tensor(f"out{s}", [CC, P, HS * 512], BF16,
                                   kind="ExternalOutput"))

    with tile.TileContext(nc) as tc:
        with tc.tile_pool(name="xg", bufs=1) as xg_pool, \
             tc.tile_pool(name="act", bufs=2) as act_pool, \
             tc.tile_pool(name="wgu", bufs=2) as wgu_pool, \
             tc.tile_pool(name="wd", bufs=1) as wd_pool, \
             tc.tile_pool(name="wvp", bufs=2) as wv_pool, \
             tc.tile_pool(name="tmp", bufs=3) as tmp_pool, \
             tc.tile_pool(name="ev", bufs=3) as ev_pool, \
             tc.tile_pool(name="psg", bufs=2, space="PSUM") as psg, \
             tc.tile_pool(name="psu", bufs=2, space="PSUM") as psu, \
             tc.tile_pool(name="psd", bufs=2, space="PSUM") as psd:
            # PE warmup (HAM unthrottle) while the first DMAs land. The warm
            # tile is the wd-pool buffer, and dmy_g/dmy_u occupy one wgu slot
            # each: guard reads of these buffers (gated on m=0 output) keep
            # the wd/wg1/wu1 prefetches from racing the critical first xg/wg
            # loads at t=0.
            warm = wd_pool.tile([P, KF * HS * 512], BF16, name="wd")
            nc.vector.memset(warm[:, :512], 0.0)
            dmy_g = wgu_pool.tile([P, KH * P], BF16, name="wg")
            nc.gpsimd.memset(dmy_g[:, :1], 0.0)
            dmy_u = wgu_pool.tile([P, KH * P], BF16, name="wu")
            nc.gpsimd.memset(dmy_u[:, :1], 0.0)
            for i in range(8):
                wps = (psg if i % 2 == 0 else psu).tile(
                    [P, 512], F32, name=("pg" if i % 2 == 0 else "pu"))
                nc.tensor.matmul(out=wps[:], lhsT=warm[:, :P],
                                 rhs=warm[:, :512], start=True, stop=True)

            for s, C in enumerate(sizes):
                xgD, wgD, wuD, wdD, wvD = ins[s]
                outD = outs[s]
                CC = (C + P - 1) // P
                CS = _slices(C)

                # token buffer, slice-major: each slice is one contiguous
                # per-partition run (full DMA line rate). Slot 0 is the
                # startup critical path: first weight tiles head the two
                # HWDGE queues, then token slices stream k-half-split across
                # both in consumption order. Later slots load while the
                # previous slot computes, so ordering there is relaxed.
                xg = xg_pool.tile([P, KH * C], BF16, name="xg")
                wg0 = wu0 = None
                if s == 0:
                    wg0 = wgu_pool.tile([P, KH * P], BF16, name="wg")
                    nc.sync.dma_start(out=wg0[:], in_=wgD[0])
                    wu0 = wgu_pool.tile([P, KH * P], BF16, name="wu")
                    nc.scalar.dma_start(out=wu0[:], in_=wuD[0])
                    for (c0, cw) in CS:
                        o, half = KH * c0, KH // 2 * cw
                        nc.sync.dma_start(out=xg[:, o:o + half],
                                          in_=xgD[:, o:o + half])
                        nc.scalar.dma_start(out=xg[:, o + half:o + KH * cw],
                                            in_=xgD[:, o + half:o + KH * cw])
                else:
                    for si, (c0, cw) in enumerate(CS):
                        o = KH * c0
                        q = nc.sync if si % 2 == 0 else nc.gpsimd
                        q.dma_start(out=xg[:, o:o + KH * cw],
                                    in_=xgD[:, o:o + KH * cw])
                wv = wv_pool.tile([P, CC], F32, name="wv")
                nc.sync.dma_start(out=wv[:], in_=wvD[:])

                actT = act_pool.tile([P, KF, C], BF16, name="actT")
                wd = None
                if s > 0:
                    # buffer reuse (bufs=1) already delays this DMA until the
                    # previous slot's down-proj has consumed its weights
                    wd = wd_pool.tile([P, KF * HS * 512], BF16, name="wd")
                    nc.scalar.dma_start(out=wd[:], in_=wdD[:])
                for m in range(MF):
                    if m == 1 and s == 0:
                        # guard reads of the dummy/warm buffers, gated on m=0
                        # output: wg1/wu1/wd prefetches (next users of those
                        # pool slots) then cannot start before real compute
                        # is underway, keeping HBM free for the critical
                        # first xg/wg loads
                        for gsrc in (dmy_g, dmy_u, warm):
                            scr = tmp_pool.tile([P, 1], F32, name="scr")
                            nc.vector.tensor_tensor(
                                out=scr[:], in0=gsrc[:, :1],
                                in1=actT[:, 0, 0:1], op=ALU.mult)
                    if m == 0 and s == 0:
                        wg, wu = wg0, wu0
                    else:
                        wg = wgu_pool.tile([P, KH * P], BF16, name="wg")
                        nc.scalar.dma_start(out=wg[:], in_=wgD[m])
                        wu = wgu_pool.tile([P, KH * P], BF16, name="wu")
                        nc.scalar.dma_start(out=wu[:], in_=wuD[m])
                    if m == 1 and s == 0:
                        # after wg1/wu1 so the 2MB wd transfer queues behind
                        # the weights m=1 actually needs
                        wd = wd_pool.tile([P, KF * HS * 512], BF16, name="wd")
                        nc.scalar.dma_start(out=wd[:], in_=wdD[:])
                    for (c0, cw) in CS:
                        o = KH * c0
                        pg = psg.tile([P, 512], F32, name="pg")[:, :cw]
                        pu = psu.tile([P, 512], F32, name="pu")[:, :cw]
                        for k in range(KH):
                            nc.tensor.matmul(
                                out=pg[:], lhsT=wg[:, k * P:(k + 1) * P],
                                rhs=xg[:, o + k * cw:o + (k + 1) * cw],
                                start=(k == 0), stop=(k == KH - 1))
                        for k in range(KH):
                            nc.tensor.matmul(
                                out=pu[:], lhsT=wu[:, k * P:(k + 1) * P],
                                rhs=xg[:, o + k * cw:o + (k + 1) * cw],
                                start=(k == 0), stop=(k == KH - 1))
                        sg = tmp_pool.tile([P, 512], F32, name="sg")[:, :cw]
                        nc.scalar.activation(out=sg[:], in_=pg[:], func=ACTF.Silu,
                                             bias=0.0, scale=1.0)
                        nc.vector.tensor_tensor(
                            out=actT[:, m, c0:c0 + cw], in0=sg[:], in1=pu[:],
                            op=ALU.mult)

                # down projection; routing weight applied at eviction; one
                # batched output DMA per 128-token chunk
                for cc in range(CC):
                    rows = min(P, C - cc * P)
                    ev = ev_pool.tile([P, HS * 512], BF16, name="ev")
                    for hs in range(HS):
                        pd = psd.tile([P, 512], F32, name="pd")
                        for k in range(KF):
                            wo = (k * HS + hs) * 512
                            nc.tensor.matmul(
                                out=pd[:rows, :],
                                lhsT=actT[:, k, cc * P:cc * P + rows],
                                rhs=wd[:, wo:wo + 512],
                                start=(k == 0), stop=(k == KF - 1))
                        nc.vector.tensor_scalar(
                            out=ev[:rows, hs * 512:(hs + 1) * 512],
                            in0=pd[:rows, :],
                            scalar1=wv[:rows, cc:cc + 1], scalar2=None,
                            op0=ALU.mult)
                    evq = nc.sync if cc % 2 == 0 else nc.scalar
                    evq.dma_start(out=outD[cc, :rows, :],
                                  in_=ev[:rows, :])
    nc.compile()
    return nc


# ---------------------------------------------------------------------------
# host routing (exact fp32 replication of the reference)
# ---------------------------------------------------------------------------
def _host_routing(x, gumbel_u, W1, b1, W2, b2, gate_w, U, alpha):
    h1 = x @ W1.T + b1
    h1 *= 1.0 / (1.0 + np.exp(-h1))                       # silu
    zl = h1 @ W2.T + b2
    g = -np.log(-np.log(gumbel_u + EPS) + EPS)
    s = (zl + g) / TAU
    s -= s.max(-1, keepdims=True)
    es = np.exp(s)
    soft = es / es.sum(-1, keepdims=True)
    hard = np.zeros_like(soft)
    hard[np.arange(T), soft.argmax(-1)] = 1.0
    z = (hard + soft) - soft                              # straight-through
    rl = x @ gate_w.T + np.float32(alpha) * (z @ U)
    rl -= rl.max(-1, keepdims=True)
    er = np.exp(rl)
    rw = er / er.sum(-1, keepdims=True)
    order = np.argsort(-rw, axis=1, kind="stable")[:, :TOP_K]
    topw = np.take_along_axis(rw, order, axis=1)
    return order, topw


def kernel(hidden_states, gumbel_u, W1, b1, W2, b2, gate_w, U, alpha, Wg, Wu, Wd):
    import time as _time

    t_start = _time.time()
    x = np.ascontiguousarray(np.asarray(hidden_states, np.float32).reshape(T, H))

    # ---- routing on host ----
    t0 = _time.time()
    order, topw = _host_routing(
        x, np.asarray(gumbel_u, np.float32),
        np.asarray(W1, np.float32), np.asarray(b1, np.float32),
        np.asarray(W2, np.float32), np.asarray(b2, np.float32),
        np.asarray(gate_w, np.float32), np.asarray(U, np.float32), alpha)
    idxs = [None] * E
    wvals = [None] * E
    tok = np.arange(T)
    for e in range(E):
        rows, cols = np.nonzero(order == e)
        idxs[e] = rows
        wvals[e] = topw[rows, cols].astype(np.float32)
    _timings["routing"] = _time.time() - t0

    # ---- pack pieces into 8 cores x nslots ----
    t0 = _time.time()
    pieces = []
    for e in range(E):
        c = len(idxs[e])
        nparts = max(1, math.ceil(c / CAP))
        base, rem = divmod(c, nparts)
        off = 0
        for i in range(nparts):
            ln = base + (1 if i < rem else 0)
            pieces.append((e, off, ln))
            off += ln

    def cost(ln):
        return 256 * ln + 16384 * math.ceil(ln / P)

    pieces.sort(key=lambda p: -p[2])
    loads = [0] * N_CORES
    assign = [[] for _ in range(N_CORES)]
    for pc in pieces:
        c = min(range(N_CORES), key=lambda i: loads[i])
        assign[c].append(pc)
        loads[c] += cost(pc[2])
    nslots = max(len(a) for a in assign)
    for a in assign:
        a.sort(key=lambda p: -p[2])
        while len(a) < nslots:
            a.append((0, 0, 0))                            # dummy slot
    sizes = [max(P, max(assign[c][i][2] for c in range(N_CORES)))
             for i in range(nslots)]

    # ---- weight/activation prep (bf16, transposed+interleaved) ----
    xT = np.ascontiguousarray(
        x.reshape(T, KH, P).transpose(2, 1, 0).astype(BF))   # [128, 16, T]
    WgB = np.asarray(Wg, np.float32).astype(BF)
    WuB = np.asarray(Wu, np.float32).astype(BF)
    WdB = np.asarray(Wd, np.float32).astype(BF)
    # wgt[e,m,p,k,j] = Wg[e, m*128+j, k*128+p]
    WgT = np.ascontiguousarray(
        WgB.reshape(E, MF, P, KH, P).transpose(0, 1, 4, 3, 2))
    WuT = np.ascontiguousarray(
        WuB.reshape(E, MF, P, KH, P).transpose(0, 1, 4, 3, 2))
    # wdt[e,p,k,hs,j] = Wd[e, hs*512+j, k*128+p]
    WdT = np.ascontiguousarray(
        WdB.reshape(E, HS, 512, KF, P).transpose(0, 4, 3, 1, 2))

    in_maps = []
    for c in range(N_CORES):
        m = {}
        for si in range(nslots):
            e, off, ln = assign[c][si]
            Csz = sizes[si]
            CC = (Csz + P - 1) // P
            xg3 = np.zeros((P, KH, Csz), BF)
            wvp = np.zeros((CC * P,), np.float32)
            if ln > 0:
                sel = idxs[e][off:off + ln]
                xg3[:, :, :ln] = xT[:, :, sel]
                wvp[:ln] = wvals[e][off:off + ln]
            # slice-major flat pack (must match kernel's per-slice offsets)
            xg = np.concatenate(
                [np.ascontiguousarray(xg3[:, :, c0:c0 + cw]).reshape(P, KH * cw)
                 for (c0, cw) in _slices(Csz)], axis=1)
            m[f"xg{si}"] = xg
            m[f"wg{si}"] = WgT[e].reshape(MF, P, KH * P)
            m[f"wu{si}"] = WuT[e].reshape(MF, P, KH * P)
            m[f"wd{si}"] = WdT[e].reshape(P, KF * HS * 512)
            m[f"wv{si}"] = np.ascontiguousarray(wvp.reshape(CC, P).T)
        in_maps.append(m)
    _timings["dispatch"] = _time.time() - t0

    t0 = _time.time()
    key = tuple(sizes)
    nc2 = _build_cache.get(key)
    if nc2 is None:
        nc2 = build_k2(sizes)
        _build_cache[key] = nc2
    _timings["k2_build"] = _time.time() - t0

    t0 = _time.time()
    res2 = run_bass_kernel_spmd(nc2, in_maps, list(range(N_CORES)), trace=TRACE)
    _timings["k2_run"] = _time.time() - t0
    if TRACE:
        _timings["k2_hw_ns"] = res2.exec_time_ns

    # ---- host combine (unshard) ----
    t0 = _time.time()
    y = np.zeros((T, H), np.float32)
    for c in range(N_CORES):
        for si in range(nslots):
            e, off, ln = assign[c][si]
            if ln == 0:
                continue
            oc = res2.results[c][f"out{si}"]             # [CC, 128, 2048] bf16
            y[idxs[e][off:off + ln]] += oc.reshape(-1, H)[:ln].astype(np.float32)
    _timings["combine"] = _time.time() - t0
    _timings["total"] = _time.time() - t_start
    return y.reshape(B, S, H)
